# revision 26
# baseline (speedup 1.0000x reference)
"""2-layer GCN encoder on 8 TRN2 NeuronCores (Bass/Tile).

Sharding: node (dst) sharding. Each core owns SLOTS windows of 128 node
slots. The host groups nodes into windows by (in-degree-from-lower-half,
in-degree-from-upper-half) so windows are degree-homogeneous, then packs
windows into slot-aligned groups of 8 (4 per half, local-search refined)
so one compiled program (identical loop bounds) serves every core with
minimal padding.

Design:
  - All gathered tables are bf16, 128 elems/row (256 B — the dma_gather
    minimum).  Layer 1 gathers RAW x rows (host-prepared, replicated
    bf16 table): no projection phase, no first AllGather.  Projection
    happens per-window after aggregation:
        z = aggT.T @ W1relT + xT_win.T @ W1rootT(+b1).
  - Layer 2 gathers h rows (bf16) after one bf16 AllGather.
  - Gathers are split into 1024-index single-packet ops (64 descriptors
    per SDMA engine = one full packet each) round-robined over the 4
    SWDGE queues — ~1.5x descriptor-drain throughput vs split packets.
  - Segment-sum via single-lane identity matmuls into PSUM (PE has
    headroom; DVE is kept off the critical path because GpSimd SWDGE
    descriptor generation contends with DVE for SBUF ports).
  - PSUM->SBUF moves and f32->bf16 casts run on the Scalar (ACT) engine.
"""

import sys

sys.path.insert(0, "/opt/trn_rl_repo")

import ml_dtypes
import numpy as np

import concourse.bacc as bacc
import concourse.bass as bass
import concourse.mybir as mybir
import concourse.tile as tile
from concourse.bass_utils import run_bass_kernel_spmd
from concourse.masks import make_identity

P = 128
NCORES = 8
BF16 = ml_dtypes.bfloat16

DEFAULT_CFG = dict(
    N=50000,   # real nodes
    F=96,      # input features
    H=128,     # hidden
    O=64,      # output features
    SLOTS=49,  # windows per core (NCORES*SLOTS*128 >= N, and N/2 <= NCORES*SLOTS*64)
    GMAX_COLS=8,  # gather-group width in columns (128 idxs each); 8 cols
                  # = 1024 idxs = the single-packet limit
    GBUFS=8,      # gather pool depth (per stream/layer tag)
)


def _derived(cfg):
    slots = cfg["SLOTS"]
    npc = slots * P              # node slots per core
    ntot = NCORES * npc          # total node slots
    half = ntot // 2             # table-half boundary (slot space)
    nhalf = cfg["N"] // 2        # real nodes per half (by original id)
    wph = half // P              # windows per half == 4*SLOTS
    assert wph == 4 * slots
    assert nhalf <= half - 1, "need at least one pad slot per half"
    assert half - 1 < 2**15, "table half must fit int16 indexing"
    return npc, ntot, half, nhalf, wph


def _chunks(cfg):
    """Slot-chunk boundaries for the chunked AllGather."""
    slots = cfg["SLOTS"]
    s_split = (slots + 1) // 2
    return [(0, s_split), (s_split, slots)]


def _table_pos(cfg, dev, slot, part):
    """Table row for (dev, slot, part): dev-major linear layout, matching
    the AllGather output order (core d's shard at rows [d*npc, (d+1)*npc))."""
    npc = cfg["SLOTS"] * P
    return np.asarray(dev).astype(np.int64) * npc + np.asarray(slot) * P + np.asarray(part)


def _group_windows(wKA0, wKB0, wKA1, wKB1, slots):
    """Group each half's windows into quads of similar (KA, KB) (greedy
    seed + local-search refinement), then pair half-0 quads with half-1
    quads (Hungarian when scipy is available, rank pairing otherwise).
    Returns (quads0, quads1, KA, KB), slot-aligned."""

    def greedy(wKA, wKB):
        order = list(np.argsort(-(wKA.astype(np.int64) + wKB)))
        remset = set(order)
        quads = []
        for _ in range(slots):
            seed = next(i for i in order if i in remset)
            remset.discard(seed)
            cands = [i for i in order if i in remset]
            cands.sort(
                key=lambda i: abs(int(wKA[i]) - int(wKA[seed]))
                + abs(int(wKB[i]) - int(wKB[seed]))
            )
            picks = cands[:3]
            for p in picks:
                remset.discard(p)
            quads.append([seed] + picks)
        return quads

    def local_search(quads, wKA, wKB, iters=60000):
        rng = np.random.default_rng(0)
        quads = [list(q) for q in quads]

        def cost(q):
            return max(int(wKA[i]) for i in q) + max(int(wKB[i]) for i in q)

        nq = len(quads)
        pairs = rng.integers(0, nq, size=(iters, 2))
        mems = rng.integers(0, 4, size=(iters, 2))
        for (a, b), (ia, ib) in zip(pairs, mems):
            if a == b:
                continue
            qa, qb = quads[a], quads[b]
            old = cost(qa) + cost(qb)
            qa[ia], qb[ib] = qb[ib], qa[ia]
            if cost(qa) + cost(qb) > old:
                qa[ia], qb[ib] = qb[ib], qa[ia]
        return quads

    q0 = local_search(greedy(wKA0, wKB0), wKA0, wKB0)
    q1 = local_search(greedy(wKA1, wKB1), wKA1, wKB1)
    c0 = [(max(int(wKA0[i]) for i in q), max(int(wKB0[i]) for i in q)) for q in q0]
    c1 = [(max(int(wKA1[i]) for i in q), max(int(wKB1[i]) for i in q)) for q in q1]
    try:
        from scipy.optimize import linear_sum_assignment

        M = np.zeros((slots, slots))
        for i, (a0, b0) in enumerate(c0):
            for j, (a1, b1) in enumerate(c1):
                M[i, j] = max(a0, a1) + max(b0, b1)
        ri, ci = linear_sum_assignment(M)
        q0 = [q0[i] for i in ri]
        q1 = [q1[j] for j in ci]
        c0 = [c0[i] for i in ri]
        c1 = [c1[j] for j in ci]
    except ImportError:
        o0 = sorted(range(slots), key=lambda i: c0[i])
        o1 = sorted(range(slots), key=lambda j: c1[j])
        q0 = [q0[i] for i in o0]
        q1 = [q1[j] for j in o1]
        c0 = [c0[i] for i in o0]
        c1 = [c1[j] for j in o1]
    KA = np.array([max(a0, a1) for (a0, _), (a1, _) in zip(c0, c1)], np.int64)
    KB = np.array([max(b0, b1) for (_, b0), (_, b1) in zip(c0, c1)], np.int64)
    # light slots first: early gather groups then span many windows, so the
    # pipeline ramps with deep lookahead at both phase starts
    order = np.argsort(KA + KB, kind="stable")
    q0 = [q0[i] for i in order]
    q1 = [q1[i] for i in order]
    KA = KA[order]
    KB = KB[order]
    return q0, q1, KA, KB


def _make_plan(src, dst, cfg):
    """Host-side planning. src/dst int32 arrays, self-loops removed."""
    N = cfg["N"]
    slots = cfg["SLOTS"]
    npc, ntot, half, nhalf, wph = _derived(cfg)

    is_a = src < nhalf
    degA = np.bincount(dst[is_a], minlength=N).astype(np.int64)
    degB = np.bincount(dst[~is_a], minlength=N).astype(np.int64)

    node_dev = np.full(N, -1, np.int32)
    node_slot = np.full(N, -1, np.int32)
    node_part = np.full(N, -1, np.int32)
    node_of = np.full((NCORES, slots, P), -1, np.int64)
    pad_pos = [None, None]  # one pad slot position per table half

    windows_h = {}
    wK = {}
    for hf in (0, 1):
        nodes = np.arange(hf * nhalf, (hf + 1) * nhalf)
        # snake order: within each degA stratum alternate degB direction so
        # stratum-boundary windows stay degB-homogeneous
        sec = np.where(degA[nodes] % 2 == 1, -degB[nodes], degB[nodes])
        order = np.lexsort((sec, degA[nodes]))
        slot_list = np.concatenate(
            [nodes[order], np.full(half - nhalf, -1, np.int64)]
        )
        windows = slot_list.reshape(wph, P)
        wmask = windows >= 0
        windows_h[hf] = windows
        wK[hf] = (
            np.where(wmask, degA[np.maximum(windows, 0)], 0).max(axis=1),
            np.where(wmask, degB[np.maximum(windows, 0)], 0).max(axis=1),
        )

    quads0, quads1, KA, KB = _group_windows(
        wK[0][0], wK[0][1], wK[1][0], wK[1][1], slots
    )
    for hf, quads in ((0, quads0), (1, quads1)):
        windows = windows_h[hf]
        for i in range(slots):
            for j, w in enumerate(quads[i]):
                d = hf * 4 + (i + j) % 4
                members = windows[w]
                node_of[d, i] = members
                real = members >= 0
                parts = np.nonzero(real)[0]
                node_dev[members[real]] = d
                node_slot[members[real]] = i
                node_part[members[real]] = parts
                if pad_pos[hf] is None and (~real).any():
                    p0 = int(np.nonzero(~real)[0][0])
                    pad_pos[hf] = int(_table_pos(cfg, d, i, p0))
    assert pad_pos[0] is not None and pad_pos[1] is not None
    assert (node_dev >= 0).all()

    # table row of each node (chunk-major-within-half layout)
    pos = _table_pos(cfg, node_dev, node_slot, node_part)
    # linear (dev, slot, part) key used only for grouping/ranking dsts
    lin = node_dev.astype(np.int64) * npc + node_slot * P + node_part

    colbaseA = np.concatenate([[0], np.cumsum(KA)])
    colbaseB = np.concatenate([[0], np.cumsum(KB)])
    LA = int(colbaseA[-1]) * P
    LB = int(colbaseB[-1]) * P

    def edge_fill(sel, colbase, Ltot, pad_val, sub):
        flat = np.full((NCORES, max(Ltot, 16)), pad_val, np.int64)
        pd = lin[dst[sel]]
        pv = pos[src[sel]] - sub
        order = np.argsort(pd, kind="stable")
        pd = pd[order]
        pv = pv[order]
        starts = np.searchsorted(pd, pd, side="left")
        rank = np.arange(len(pd)) - starts
        dev = pd // npc
        slot = (pd % npc) // P
        part = pd % P
        fpos = (colbase[slot] + rank) * P + part
        flat[dev, fpos] = pv
        assert flat.min() >= 0 and flat.max() < half
        # wrap: element i -> [i % 16, i // 16], then replicate block to 128 rows
        wrapped = flat.reshape(NCORES, -1, 16).transpose(0, 2, 1)
        return np.tile(wrapped, (1, 8, 1)).astype(np.int16)

    idxA = edge_fill(is_a, colbaseA, LA, pad_pos[0], 0)
    idxB = edge_fill(~is_a, colbaseB, LB, pad_pos[1] - half, half)

    def make_groups(colbase):
        # uniform column-granular groups of GMAX_COLS columns; a slot's
        # columns may span adjacent groups
        gmax = cfg["GMAX_COLS"]
        total = int(colbase[-1])
        groups = []
        c0 = 0
        while c0 < total:
            groups.append((c0, min(c0 + gmax, total)))
            c0 += gmax
        return groups

    groupsA = make_groups(colbaseA)
    groupsB = make_groups(colbaseB)

    meta = dict(
        cfg=dict(cfg),
        KA=[int(v) for v in KA],
        KB=[int(v) for v in KB],
        colbaseA=[int(v) for v in colbaseA],
        colbaseB=[int(v) for v in colbaseB],
        LA=max(LA, 16),
        LB=max(LB, 16),
        groupsA=groupsA,
        groupsB=groupsB,
    )
    return dict(
        meta=meta,
        node_dev=node_dev,
        node_slot=node_slot,
        node_part=node_part,
        node_of=node_of,
        idxA=idxA,
        idxB=idxB,
    )


def _make_in_maps(plan, cfg, x, W1_rel, b1, W1_root, W2_rel, b2, W2_root):
    F, H, O = cfg["F"], cfg["H"], cfg["O"]
    slots = cfg["SLOTS"]
    npc, ntot, _, _, _ = _derived(cfg)
    node_of = plan["node_of"]

    # x table in table-position order, feature-padded to 128, bf16,
    # replicated; pad rows stay zero.
    x_tab = np.zeros((ntot, P), BF16)
    tp = _table_pos(
        cfg, plan["node_dev"], plan["node_slot"], plan["node_part"]
    )
    x_tab[tp, :F] = x.astype(BF16)

    w1relT = np.zeros((P, H), np.float32)
    w1relT[:F] = W1_rel.T
    w1rootT = np.zeros((F + 1, H), np.float32)
    w1rootT[:F] = W1_root.T
    w1rootT[F] = b1
    w2relT = np.ascontiguousarray(W2_rel.T, dtype=np.float32)
    w2rootT = np.ascontiguousarray(W2_root.T, dtype=np.float32)
    b2bc = np.ascontiguousarray(np.broadcast_to(b2, (P, O)), dtype=np.float32)

    in_maps = []
    for d in range(NCORES):
        members = node_of[d].reshape(-1)  # [npc]
        real = members >= 0
        xT = np.zeros((F + 1, npc), BF16)
        xT[:F, real] = x[members[real]].T.astype(BF16)
        xT[F] = 1.0
        valid = np.zeros((P, slots), np.float32)
        valid[:, :] = real.reshape(slots, P).T
        in_maps.append(
            dict(
                x_tab=x_tab,
                xT=xT,
                w1relT=w1relT.astype(BF16),
                w1rootT=w1rootT.astype(BF16),
                w2relT=w2relT.astype(BF16),
                w2rootT=w2rootT.astype(BF16),
                b2bc=b2bc,
                valid=valid,
                idxA=np.ascontiguousarray(plan["idxA"][d]),
                idxB=np.ascontiguousarray(plan["idxB"][d]),
            )
        )
    return in_maps


def _build_nc(meta):
    cfg = meta["cfg"]
    F, H, O = cfg["F"], cfg["H"], cfg["O"]
    slots = cfg["SLOTS"]
    npc, ntot, half, _, _ = _derived(cfg)
    KA, KB = meta["KA"], meta["KB"]
    f32 = mybir.dt.float32
    bf16 = mybir.dt.bfloat16
    i16 = mybir.dt.int16
    RG = [list(range(NCORES))]

    nc = bacc.Bacc(
        "TRN2",
        target_bir_lowering=False,
        debug=False,
        num_devices=NCORES,
        num_swdge_queues=4,
    )
    xtab_d = nc.dram_tensor("x_tab", [ntot, P], bf16, kind="ExternalInput")
    xT_d = nc.dram_tensor("xT", [F + 1, npc], bf16, kind="ExternalInput")
    w1r_d = nc.dram_tensor("w1relT", [P, H], bf16, kind="ExternalInput")
    w1o_d = nc.dram_tensor("w1rootT", [F + 1, H], bf16, kind="ExternalInput")
    w2r_d = nc.dram_tensor("w2relT", [H, O], bf16, kind="ExternalInput")
    w2o_d = nc.dram_tensor("w2rootT", [H, O], bf16, kind="ExternalInput")
    b2_d = nc.dram_tensor("b2bc", [P, O], f32, kind="ExternalInput")
    vld_d = nc.dram_tensor("valid", [P, slots], f32, kind="ExternalInput")
    ixA_d = nc.dram_tensor("idxA", [P, meta["LA"] // 16], i16, kind="ExternalInput")
    ixB_d = nc.dram_tensor("idxB", [P, meta["LB"] // 16], i16, kind="ExternalInput")
    out_d = nc.dram_tensor("out", [npc, O], f32, kind="ExternalOutput")

    h_loc = nc.dram_tensor("h_loc", [npc, H], bf16)
    h_full = nc.dram_tensor("h_full", [ntot, H], bf16, addr_space="Shared")

    with tile.TileContext(nc) as tc:
        with (
            tc.tile_pool(name="const", bufs=1) as cp,
            tc.tile_pool(name="work", bufs=3) as wp,
            tc.tile_pool(name="gath", bufs=cfg["GBUFS"]) as gp,
            tc.tile_pool(name="psacc", bufs=4, space="PSUM") as pa,
            tc.tile_pool(name="psum", bufs=2, space="PSUM") as pp,
        ):
            def load_const(tag, dram, shape, dtype=f32):
                t = cp.tile(shape, dtype, tag=tag)
                nc.sync.dma_start(out=t[:], in_=dram[:])
                return t

            xt = load_const("xt", xT_d, [F + 1, npc], bf16)
            w1r = load_const("w1r", w1r_d, [P, H], bf16)
            w1o = load_const("w1o", w1o_d, [F + 1, H], bf16)
            w2r = load_const("w2r", w2r_d, [H, O], bf16)
            w2o = load_const("w2o", w2o_d, [H, O], bf16)
            b2 = load_const("b2", b2_d, [P, O])
            vld = load_const("vld", vld_d, [P, slots])
            ixA = load_const("ixA", ixA_d, [P, meta["LA"] // 16], i16)
            ixB = load_const("ixB", ixB_d, [P, meta["LB"] // 16], i16)
            ident = cp.tile([P, P], bf16, tag="ident")
            make_identity(nc, ident[:])
            hT = cp.tile([P, npc], bf16, tag="hT")

            emitted = {}
            gq = [0]  # round-robin SWDGE queue for gathers
            gmax = cfg["GMAX_COLS"]
            colbase = (meta["colbaseA"], meta["colbaseB"])
            groups = (meta["groupsA"], meta["groupsB"])

            def gtile(layer, stream, gid):
                key = (layer, stream, gid)
                if key not in emitted:
                    c0, c1 = groups[stream][gid]
                    L = (c1 - c0) * P
                    table = xtab_d if layer == 1 else h_full
                    half_ap = table[:half, :] if stream == 0 else table[half:, :]
                    ix = ixA if stream == 0 else ixB
                    t = gp.tile([P, (c1 - c0) * P], bf16, tag=f"g{stream}l{layer}")
                    nc.gpsimd.dma_gather(
                        out_ap=t[:].rearrange("p (c e) -> p c e", e=P),
                        in_ap=half_ap,
                        idxs_ap=ix[:, c0 * 8 : c1 * 8],
                        num_idxs=L,
                        num_idxs_reg=L,
                        elem_size=P,
                        # >~1024 idxs in one packet overflows the packet
                        # limit on HW (sim doesn't model it) — split packets
                        single_packet=(L <= 1024),
                        queue_num=gq[0],
                    )
                    gq[0] = (gq[0] + 1) % 4
                    emitted[key] = t
                return emitted[key]

            def aggregate(layer, s):
                """Single-lane identity-matmul segment-sum of the gathered
                bf16 rows for window s, then transpose; returns the [P, P]
                bf16 aggT (feature on partition) in SBUF, or None if the
                window has no in-edges."""
                cols = []
                for stream in (0, 1):
                    Ks = (KA if stream == 0 else KB)[s]
                    base = colbase[stream][s]
                    for c in range(base, base + Ks):
                        t = gtile(layer, stream, c // gmax)
                        cols.append((t, c % gmax))
                if not cols:
                    return None
                ps1 = pa.tile([P, P], f32, tag="ps_acc")
                for i, (t, c) in enumerate(cols):
                    nc.tensor.matmul(
                        ps1[:],
                        lhsT=ident[:],
                        rhs=t[:, c * P : (c + 1) * P],
                        start=(i == 0),
                        stop=(i == len(cols) - 1),
                    )
                agg = wp.tile([P, P], bf16, tag="agg")
                nc.scalar.copy(agg[:], ps1[:])
                pt = pp.tile([P, P], bf16, tag="ps_tr")
                nc.tensor.transpose(pt[:], agg[:], ident[:])
                aggT = wp.tile([P, P], bf16, tag="aggT")
                nc.scalar.copy(aggT[:], pt[:])
                return aggT

            # ---- layer 1
            for s in range(slots):
                aggT = aggregate(1, s)
                pz = pp.tile([P, H], f32, tag="ps_z")
                if aggT is not None:
                    nc.tensor.matmul(
                        pz[:], lhsT=aggT[:], rhs=w1r[:],
                        start=True, stop=False,
                    )
                nc.tensor.matmul(
                    pz[:],
                    lhsT=xt[:, s * P : (s + 1) * P],
                    rhs=w1o[:],
                    start=(aggT is None),
                    stop=True,
                )
                h = wp.tile([P, H], bf16, tag="h")
                # relu(z)*v == relu(z*v) for v in {0,1}: fold the pad-node
                # mask into the activation's per-partition scale
                nc.scalar.activation(
                    h[:],
                    pz[:],
                    mybir.ActivationFunctionType.Relu,
                    scale=vld[:, s : s + 1],
                )
                nc.sync.dma_start(out=h_loc[s * P : (s + 1) * P, :], in_=h[:])
                pt = pp.tile([P, P], bf16, tag="ps_tr")
                nc.tensor.transpose(pt[:], h[:], ident[:])
                nc.scalar.copy(hT[:, s * P : (s + 1) * P], pt[:])
            nc.gpsimd.collective_compute(
                "AllGather",
                mybir.AluOpType.bypass,
                replica_groups=RG,
                ins=[h_loc[:]],
                outs=[h_full[:]],
            )

            # ---- layer 2
            for s in range(slots):
                aggT = aggregate(2, s)
                pzf = pp.tile([P, H], f32, tag="ps_z")
                pz = pzf[:, :O]
                if aggT is not None:
                    nc.tensor.matmul(
                        pz, lhsT=aggT[:], rhs=w2r[:],
                        start=True, stop=False,
                    )
                nc.tensor.matmul(
                    pz,
                    lhsT=hT[:, s * P : (s + 1) * P],
                    rhs=w2o[:],
                    start=(aggT is None),
                    stop=True,
                )
                ot = wp.tile([P, O], f32, tag="small")
                nc.vector.tensor_tensor(
                    out=ot[:], in0=pz, in1=b2[:], op=mybir.AluOpType.add
                )
                nc.sync.dma_start(out=out_d[s * P : (s + 1) * P, :], in_=ot[:])

    nc.compile()
    return nc


_NC_CACHE = {}


def _meta_key(meta):
    return repr(
        (
            meta["cfg"],
            meta["KA"],
            meta["KB"],
            meta["groupsA"],
            meta["groupsB"],
        )
    )


def _run(inputs, cfg=None, trace=False):
    cfg = dict(DEFAULT_CFG if cfg is None else cfg)
    x = np.ascontiguousarray(np.asarray(inputs["x"], np.float32))
    ei = np.asarray(inputs["edge_index"])
    src = ei[0].astype(np.int64)
    dst = ei[1].astype(np.int64)
    keep = src != dst
    src = src[keep].astype(np.int32)
    dst = dst[keep].astype(np.int32)

    plan = _make_plan(src, dst, cfg)
    key = _meta_key(plan["meta"])
    if key not in _NC_CACHE:
        _NC_CACHE[key] = _build_nc(plan["meta"])
    nc = _NC_CACHE[key]

    in_maps = _make_in_maps(
        plan,
        cfg,
        x,
        np.asarray(inputs["W1_rel"], np.float32),
        np.asarray(inputs["b1"], np.float32),
        np.asarray(inputs["W1_root"], np.float32),
        np.asarray(inputs["W2_rel"], np.float32),
        np.asarray(inputs["b2"], np.float32),
        np.asarray(inputs["W2_root"], np.float32),
    )
    res = run_bass_kernel_spmd(
        nc, in_maps, list(range(NCORES)), trace=trace
    )

    N, O = cfg["N"], cfg["O"]
    out = np.empty((N, O), np.float32)
    local = plan["node_slot"] * P + plan["node_part"]
    for d in range(NCORES):
        sel = plan["node_dev"] == d
        out[sel] = res.results[d]["out"][local[sel]]
    return out, res


def kernel(**inputs) -> np.ndarray:
    out, _ = _run(inputs)
    return out


# revision 27
# speedup vs baseline: 1.0894x; 1.0894x over previous
"""2-layer GCN encoder on 8 TRN2 NeuronCores (Bass/Tile).

Sharding: node (dst) sharding. Each core owns SLOTS windows of 128 node
slots. The host groups nodes into windows by (in-degree-from-lower-half,
in-degree-from-upper-half) so windows are degree-homogeneous, then packs
windows into slot-aligned groups of 8 (4 per half, local-search refined)
so one compiled program (identical loop bounds) serves every core with
minimal padding.

Design:
  - All gathered tables are bf16, 128 elems/row (256 B — the dma_gather
    minimum).  Layer 1 gathers RAW x rows (host-prepared, replicated
    bf16 table): no projection phase, no first AllGather.  Projection
    happens per-window after aggregation:
        z = aggT.T @ W1relT + xT_win.T @ W1rootT(+b1).
  - Layer 2 gathers h rows (bf16) after one bf16 AllGather.
  - Gathers are split into 1024-index single-packet ops (64 descriptors
    per SDMA engine = one full packet each) round-robined over the 4
    SWDGE queues — ~1.5x descriptor-drain throughput vs split packets.
  - Segment-sum via single-lane identity matmuls into PSUM (PE has
    headroom; DVE is kept off the critical path because GpSimd SWDGE
    descriptor generation contends with DVE for SBUF ports).
  - PSUM->SBUF moves and f32->bf16 casts run on the Scalar (ACT) engine.
"""

import sys

sys.path.insert(0, "/opt/trn_rl_repo")

import ml_dtypes
import numpy as np

import concourse.bacc as bacc
import concourse.bass as bass
import concourse.mybir as mybir
import concourse.tile as tile
from concourse.bass_utils import run_bass_kernel_spmd
from concourse.masks import make_identity

P = 128
NCORES = 8
BF16 = ml_dtypes.bfloat16

DEFAULT_CFG = dict(
    N=50000,   # real nodes
    F=96,      # input features
    H=128,     # hidden
    O=64,      # output features
    SLOTS=49,  # windows per core (NCORES*SLOTS*128 >= N, and N/2 <= NCORES*SLOTS*64)
    GMAX_COLS=8,  # gather-group width in columns (128 idxs each); 8 cols
                  # = 1024 idxs = the single-packet limit
    GBUFS=8,      # gather pool depth (per stream/layer tag)
)


def _derived(cfg):
    slots = cfg["SLOTS"]
    npc = slots * P              # node slots per core
    ntot = NCORES * npc          # total node slots
    half = ntot // 2             # table-half boundary (slot space)
    nhalf = cfg["N"] // 2        # real nodes per half (by original id)
    wph = half // P              # windows per half == 4*SLOTS
    assert wph == 4 * slots
    assert nhalf <= half - 1, "need at least one pad slot per half"
    assert half - 1 < 2**15, "table half must fit int16 indexing"
    return npc, ntot, half, nhalf, wph


def _chunks(cfg):
    """Slot-chunk boundaries for the chunked AllGather."""
    slots = cfg["SLOTS"]
    s_split = (slots + 1) // 2
    return [(0, s_split), (s_split, slots)]


def _table_pos(cfg, dev, slot, part):
    """Table row for (dev, slot, part): dev-major linear layout, matching
    the AllGather output order (core d's shard at rows [d*npc, (d+1)*npc))."""
    npc = cfg["SLOTS"] * P
    return np.asarray(dev).astype(np.int64) * npc + np.asarray(slot) * P + np.asarray(part)


def _group_windows(wKA0, wKB0, wKA1, wKB1, slots):
    """Group each half's windows into quads of similar (KA, KB) (greedy
    seed + local-search refinement), then pair half-0 quads with half-1
    quads (Hungarian when scipy is available, rank pairing otherwise).
    Returns (quads0, quads1, KA, KB), slot-aligned."""

    def greedy(wKA, wKB):
        order = list(np.argsort(-(wKA.astype(np.int64) + wKB)))
        remset = set(order)
        quads = []
        for _ in range(slots):
            seed = next(i for i in order if i in remset)
            remset.discard(seed)
            cands = [i for i in order if i in remset]
            cands.sort(
                key=lambda i: abs(int(wKA[i]) - int(wKA[seed]))
                + abs(int(wKB[i]) - int(wKB[seed]))
            )
            picks = cands[:3]
            for p in picks:
                remset.discard(p)
            quads.append([seed] + picks)
        return quads

    def local_search(quads, wKA, wKB, iters=60000):
        rng = np.random.default_rng(0)
        quads = [list(q) for q in quads]

        def cost(q):
            return max(int(wKA[i]) for i in q) + max(int(wKB[i]) for i in q)

        nq = len(quads)
        pairs = rng.integers(0, nq, size=(iters, 2))
        mems = rng.integers(0, 4, size=(iters, 2))
        for (a, b), (ia, ib) in zip(pairs, mems):
            if a == b:
                continue
            qa, qb = quads[a], quads[b]
            old = cost(qa) + cost(qb)
            qa[ia], qb[ib] = qb[ib], qa[ia]
            if cost(qa) + cost(qb) > old:
                qa[ia], qb[ib] = qb[ib], qa[ia]
        return quads

    q0 = local_search(greedy(wKA0, wKB0), wKA0, wKB0)
    q1 = local_search(greedy(wKA1, wKB1), wKA1, wKB1)
    c0 = [(max(int(wKA0[i]) for i in q), max(int(wKB0[i]) for i in q)) for q in q0]
    c1 = [(max(int(wKA1[i]) for i in q), max(int(wKB1[i]) for i in q)) for q in q1]
    try:
        from scipy.optimize import linear_sum_assignment

        M = np.zeros((slots, slots))
        for i, (a0, b0) in enumerate(c0):
            for j, (a1, b1) in enumerate(c1):
                M[i, j] = max(a0, a1) + max(b0, b1)
        ri, ci = linear_sum_assignment(M)
        q0 = [q0[i] for i in ri]
        q1 = [q1[j] for j in ci]
        c0 = [c0[i] for i in ri]
        c1 = [c1[j] for j in ci]
    except ImportError:
        o0 = sorted(range(slots), key=lambda i: c0[i])
        o1 = sorted(range(slots), key=lambda j: c1[j])
        q0 = [q0[i] for i in o0]
        q1 = [q1[j] for j in o1]
        c0 = [c0[i] for i in o0]
        c1 = [c1[j] for j in o1]
    KA = np.array([max(a0, a1) for (a0, _), (a1, _) in zip(c0, c1)], np.int64)
    KB = np.array([max(b0, b1) for (_, b0), (_, b1) in zip(c0, c1)], np.int64)
    return q0, q1, KA, KB


def _make_plan(src, dst, cfg):
    """Host-side planning. src/dst int32 arrays, self-loops removed."""
    N = cfg["N"]
    slots = cfg["SLOTS"]
    npc, ntot, half, nhalf, wph = _derived(cfg)

    is_a = src < nhalf
    degA = np.bincount(dst[is_a], minlength=N).astype(np.int64)
    degB = np.bincount(dst[~is_a], minlength=N).astype(np.int64)

    node_dev = np.full(N, -1, np.int32)
    node_slot = np.full(N, -1, np.int32)
    node_part = np.full(N, -1, np.int32)
    node_of = np.full((NCORES, slots, P), -1, np.int64)
    pad_pos = [None, None]  # one pad slot position per table half

    windows_h = {}
    wK = {}
    for hf in (0, 1):
        nodes = np.arange(hf * nhalf, (hf + 1) * nhalf)
        # snake order: within each degA stratum alternate degB direction so
        # stratum-boundary windows stay degB-homogeneous
        sec = np.where(degA[nodes] % 2 == 1, -degB[nodes], degB[nodes])
        order = np.lexsort((sec, degA[nodes]))
        slot_list = np.concatenate(
            [nodes[order], np.full(half - nhalf, -1, np.int64)]
        )
        windows = slot_list.reshape(wph, P)
        wmask = windows >= 0
        windows_h[hf] = windows
        wK[hf] = (
            np.where(wmask, degA[np.maximum(windows, 0)], 0).max(axis=1),
            np.where(wmask, degB[np.maximum(windows, 0)], 0).max(axis=1),
        )

    quads0, quads1, KA, KB = _group_windows(
        wK[0][0], wK[0][1], wK[1][0], wK[1][1], slots
    )
    for hf, quads in ((0, quads0), (1, quads1)):
        windows = windows_h[hf]
        for i in range(slots):
            for j, w in enumerate(quads[i]):
                d = hf * 4 + (i + j) % 4
                members = windows[w]
                node_of[d, i] = members
                real = members >= 0
                parts = np.nonzero(real)[0]
                node_dev[members[real]] = d
                node_slot[members[real]] = i
                node_part[members[real]] = parts
                if pad_pos[hf] is None and (~real).any():
                    p0 = int(np.nonzero(~real)[0][0])
                    pad_pos[hf] = int(_table_pos(cfg, d, i, p0))
    assert pad_pos[0] is not None and pad_pos[1] is not None
    assert (node_dev >= 0).all()

    # table row of each node (chunk-major-within-half layout)
    pos = _table_pos(cfg, node_dev, node_slot, node_part)
    # linear (dev, slot, part) key used only for grouping/ranking dsts
    lin = node_dev.astype(np.int64) * npc + node_slot * P + node_part

    colbaseA = np.concatenate([[0], np.cumsum(KA)])
    colbaseB = np.concatenate([[0], np.cumsum(KB)])
    LA = int(colbaseA[-1]) * P
    LB = int(colbaseB[-1]) * P

    def edge_fill(sel, colbase, Ltot, pad_val, sub):
        flat = np.full((NCORES, max(Ltot, 16)), pad_val, np.int64)
        pd = lin[dst[sel]]
        pv = pos[src[sel]] - sub
        order = np.argsort(pd, kind="stable")
        pd = pd[order]
        pv = pv[order]
        starts = np.searchsorted(pd, pd, side="left")
        rank = np.arange(len(pd)) - starts
        dev = pd // npc
        slot = (pd % npc) // P
        part = pd % P
        fpos = (colbase[slot] + rank) * P + part
        flat[dev, fpos] = pv
        assert flat.min() >= 0 and flat.max() < half
        # wrap: element i -> [i % 16, i // 16], then replicate block to 128 rows
        wrapped = flat.reshape(NCORES, -1, 16).transpose(0, 2, 1)
        return np.tile(wrapped, (1, 8, 1)).astype(np.int16)

    idxA = edge_fill(is_a, colbaseA, LA, pad_pos[0], 0)
    idxB = edge_fill(~is_a, colbaseB, LB, pad_pos[1] - half, half)

    def make_groups(colbase):
        # uniform column-granular groups of GMAX_COLS columns; a slot's
        # columns may span adjacent groups
        gmax = cfg["GMAX_COLS"]
        total = int(colbase[-1])
        groups = []
        c0 = 0
        while c0 < total:
            groups.append((c0, min(c0 + gmax, total)))
            c0 += gmax
        return groups

    groupsA = make_groups(colbaseA)
    groupsB = make_groups(colbaseB)

    meta = dict(
        cfg=dict(cfg),
        KA=[int(v) for v in KA],
        KB=[int(v) for v in KB],
        colbaseA=[int(v) for v in colbaseA],
        colbaseB=[int(v) for v in colbaseB],
        LA=max(LA, 16),
        LB=max(LB, 16),
        groupsA=groupsA,
        groupsB=groupsB,
    )
    return dict(
        meta=meta,
        node_dev=node_dev,
        node_slot=node_slot,
        node_part=node_part,
        node_of=node_of,
        idxA=idxA,
        idxB=idxB,
    )


def _make_in_maps(plan, cfg, x, W1_rel, b1, W1_root, W2_rel, b2, W2_root):
    F, H, O = cfg["F"], cfg["H"], cfg["O"]
    slots = cfg["SLOTS"]
    npc, ntot, _, _, _ = _derived(cfg)
    node_of = plan["node_of"]

    # x table in table-position order, feature-padded to 128, bf16,
    # replicated; pad rows stay zero.
    x_tab = np.zeros((ntot, P), BF16)
    tp = _table_pos(
        cfg, plan["node_dev"], plan["node_slot"], plan["node_part"]
    )
    x_tab[tp, :F] = x.astype(BF16)

    w1relT = np.zeros((P, H), np.float32)
    w1relT[:F] = W1_rel.T
    w1rootT = np.zeros((F + 1, H), np.float32)
    w1rootT[:F] = W1_root.T
    w1rootT[F] = b1
    w2relT = np.ascontiguousarray(W2_rel.T, dtype=np.float32)
    w2rootT = np.ascontiguousarray(W2_root.T, dtype=np.float32)
    b2bc = np.ascontiguousarray(np.broadcast_to(b2, (P, O)), dtype=np.float32)

    in_maps = []
    for d in range(NCORES):
        members = node_of[d].reshape(-1)  # [npc]
        real = members >= 0
        xT = np.zeros((F + 1, npc), BF16)
        xT[:F, real] = x[members[real]].T.astype(BF16)
        xT[F] = 1.0
        valid = np.zeros((P, slots), np.float32)
        valid[:, :] = real.reshape(slots, P).T
        in_maps.append(
            dict(
                x_tab=x_tab,
                xT=xT,
                w1relT=w1relT.astype(BF16),
                w1rootT=w1rootT.astype(BF16),
                w2relT=w2relT.astype(BF16),
                w2rootT=w2rootT.astype(BF16),
                b2bc=b2bc,
                valid=valid,
                idxA=np.ascontiguousarray(plan["idxA"][d]),
                idxB=np.ascontiguousarray(plan["idxB"][d]),
            )
        )
    return in_maps


def _build_nc(meta):
    cfg = meta["cfg"]
    F, H, O = cfg["F"], cfg["H"], cfg["O"]
    slots = cfg["SLOTS"]
    npc, ntot, half, _, _ = _derived(cfg)
    KA, KB = meta["KA"], meta["KB"]
    f32 = mybir.dt.float32
    bf16 = mybir.dt.bfloat16
    i16 = mybir.dt.int16
    RG = [list(range(NCORES))]

    nc = bacc.Bacc(
        "TRN2",
        target_bir_lowering=False,
        debug=False,
        num_devices=NCORES,
        num_swdge_queues=4,
    )
    xtab_d = nc.dram_tensor("x_tab", [ntot, P], bf16, kind="ExternalInput")
    xT_d = nc.dram_tensor("xT", [F + 1, npc], bf16, kind="ExternalInput")
    w1r_d = nc.dram_tensor("w1relT", [P, H], bf16, kind="ExternalInput")
    w1o_d = nc.dram_tensor("w1rootT", [F + 1, H], bf16, kind="ExternalInput")
    w2r_d = nc.dram_tensor("w2relT", [H, O], bf16, kind="ExternalInput")
    w2o_d = nc.dram_tensor("w2rootT", [H, O], bf16, kind="ExternalInput")
    b2_d = nc.dram_tensor("b2bc", [P, O], f32, kind="ExternalInput")
    vld_d = nc.dram_tensor("valid", [P, slots], f32, kind="ExternalInput")
    ixA_d = nc.dram_tensor("idxA", [P, meta["LA"] // 16], i16, kind="ExternalInput")
    ixB_d = nc.dram_tensor("idxB", [P, meta["LB"] // 16], i16, kind="ExternalInput")
    out_d = nc.dram_tensor("out", [npc, O], f32, kind="ExternalOutput")

    h_loc = nc.dram_tensor("h_loc", [npc, H], bf16)
    h_full = nc.dram_tensor("h_full", [ntot, H], bf16, addr_space="Shared")

    with tile.TileContext(nc) as tc:
        with (
            tc.tile_pool(name="const", bufs=1) as cp,
            tc.tile_pool(name="work", bufs=3) as wp,
            tc.tile_pool(name="gath", bufs=cfg["GBUFS"]) as gp,
            tc.tile_pool(name="psacc", bufs=4, space="PSUM") as pa,
            tc.tile_pool(name="psum", bufs=2, space="PSUM") as pp,
        ):
            def load_const(tag, dram, shape, dtype=f32):
                t = cp.tile(shape, dtype, tag=tag)
                nc.sync.dma_start(out=t[:], in_=dram[:])
                return t

            xt = load_const("xt", xT_d, [F + 1, npc], bf16)
            w1r = load_const("w1r", w1r_d, [P, H], bf16)
            w1o = load_const("w1o", w1o_d, [F + 1, H], bf16)
            w2r = load_const("w2r", w2r_d, [H, O], bf16)
            w2o = load_const("w2o", w2o_d, [H, O], bf16)
            b2 = load_const("b2", b2_d, [P, O])
            vld = load_const("vld", vld_d, [P, slots])
            ixA = load_const("ixA", ixA_d, [P, meta["LA"] // 16], i16)
            ixB = load_const("ixB", ixB_d, [P, meta["LB"] // 16], i16)
            ident = cp.tile([P, P], bf16, tag="ident")
            make_identity(nc, ident[:])
            hT = cp.tile([P, npc], bf16, tag="hT")

            emitted = {}
            gq = [0]  # round-robin SWDGE queue for gathers
            gmax = cfg["GMAX_COLS"]
            colbase = (meta["colbaseA"], meta["colbaseB"])
            groups = (meta["groupsA"], meta["groupsB"])

            def gtile(layer, stream, gid):
                key = (layer, stream, gid)
                if key not in emitted:
                    c0, c1 = groups[stream][gid]
                    L = (c1 - c0) * P
                    table = xtab_d if layer == 1 else h_full
                    half_ap = table[:half, :] if stream == 0 else table[half:, :]
                    ix = ixA if stream == 0 else ixB
                    t = gp.tile([P, (c1 - c0) * P], bf16, tag=f"g{stream}l{layer}")
                    nc.gpsimd.dma_gather(
                        out_ap=t[:].rearrange("p (c e) -> p c e", e=P),
                        in_ap=half_ap,
                        idxs_ap=ix[:, c0 * 8 : c1 * 8],
                        num_idxs=L,
                        num_idxs_reg=L,
                        elem_size=P,
                        # >~1024 idxs in one packet overflows the packet
                        # limit on HW (sim doesn't model it) — split packets
                        single_packet=(L <= 1024),
                        queue_num=gq[0],
                    )
                    gq[0] = (gq[0] + 1) % 4
                    emitted[key] = t
                return emitted[key]

            def aggregate(layer, s):
                """Single-lane identity-matmul segment-sum of the gathered
                bf16 rows for window s, then transpose; returns the [P, P]
                bf16 aggT (feature on partition) in SBUF, or None if the
                window has no in-edges."""
                cols = []
                for stream in (0, 1):
                    Ks = (KA if stream == 0 else KB)[s]
                    base = colbase[stream][s]
                    for c in range(base, base + Ks):
                        t = gtile(layer, stream, c // gmax)
                        cols.append((t, c % gmax))
                if not cols:
                    return None
                ps1 = pa.tile([P, P], f32, tag="ps_acc")
                for i, (t, c) in enumerate(cols):
                    nc.tensor.matmul(
                        ps1[:],
                        lhsT=ident[:],
                        rhs=t[:, c * P : (c + 1) * P],
                        start=(i == 0),
                        stop=(i == len(cols) - 1),
                    )
                agg = wp.tile([P, P], bf16, tag="agg")
                nc.scalar.copy(agg[:], ps1[:])
                pt = pp.tile([P, P], bf16, tag="ps_tr")
                nc.tensor.transpose(pt[:], agg[:], ident[:])
                aggT = wp.tile([P, P], bf16, tag="aggT")
                nc.scalar.copy(aggT[:], pt[:])
                return aggT

            # ---- layer 1
            for s in range(slots):
                aggT = aggregate(1, s)
                pz = pp.tile([P, H], f32, tag="ps_z")
                if aggT is not None:
                    nc.tensor.matmul(
                        pz[:], lhsT=aggT[:], rhs=w1r[:],
                        start=True, stop=False,
                    )
                nc.tensor.matmul(
                    pz[:],
                    lhsT=xt[:, s * P : (s + 1) * P],
                    rhs=w1o[:],
                    start=(aggT is None),
                    stop=True,
                )
                h = wp.tile([P, H], bf16, tag="h")
                # relu(z)*v == relu(z*v) for v in {0,1}: fold the pad-node
                # mask into the activation's per-partition scale
                nc.scalar.activation(
                    h[:],
                    pz[:],
                    mybir.ActivationFunctionType.Relu,
                    scale=vld[:, s : s + 1],
                )
                nc.sync.dma_start(out=h_loc[s * P : (s + 1) * P, :], in_=h[:])
                pt = pp.tile([P, P], bf16, tag="ps_tr")
                nc.tensor.transpose(pt[:], h[:], ident[:])
                nc.scalar.copy(hT[:, s * P : (s + 1) * P], pt[:])
            nc.gpsimd.collective_compute(
                "AllGather",
                mybir.AluOpType.bypass,
                replica_groups=RG,
                ins=[h_loc[:]],
                outs=[h_full[:]],
            )

            # ---- layer 2
            for s in range(slots):
                aggT = aggregate(2, s)
                pzf = pp.tile([P, H], f32, tag="ps_z")
                pz = pzf[:, :O]
                if aggT is not None:
                    nc.tensor.matmul(
                        pz, lhsT=aggT[:], rhs=w2r[:],
                        start=True, stop=False,
                    )
                nc.tensor.matmul(
                    pz,
                    lhsT=hT[:, s * P : (s + 1) * P],
                    rhs=w2o[:],
                    start=(aggT is None),
                    stop=True,
                )
                ot = wp.tile([P, O], f32, tag="small")
                nc.vector.tensor_tensor(
                    out=ot[:], in0=pz, in1=b2[:], op=mybir.AluOpType.add
                )
                nc.sync.dma_start(out=out_d[s * P : (s + 1) * P, :], in_=ot[:])

    nc.compile()
    return nc


_NC_CACHE = {}


def _meta_key(meta):
    return repr(
        (
            meta["cfg"],
            meta["KA"],
            meta["KB"],
            meta["groupsA"],
            meta["groupsB"],
        )
    )


def _run(inputs, cfg=None, trace=False):
    cfg = dict(DEFAULT_CFG if cfg is None else cfg)
    x = np.ascontiguousarray(np.asarray(inputs["x"], np.float32))
    ei = np.asarray(inputs["edge_index"])
    src = ei[0].astype(np.int64)
    dst = ei[1].astype(np.int64)
    keep = src != dst
    src = src[keep].astype(np.int32)
    dst = dst[keep].astype(np.int32)

    plan = _make_plan(src, dst, cfg)
    key = _meta_key(plan["meta"])
    if key not in _NC_CACHE:
        _NC_CACHE[key] = _build_nc(plan["meta"])
    nc = _NC_CACHE[key]

    in_maps = _make_in_maps(
        plan,
        cfg,
        x,
        np.asarray(inputs["W1_rel"], np.float32),
        np.asarray(inputs["b1"], np.float32),
        np.asarray(inputs["W1_root"], np.float32),
        np.asarray(inputs["W2_rel"], np.float32),
        np.asarray(inputs["b2"], np.float32),
        np.asarray(inputs["W2_root"], np.float32),
    )
    res = run_bass_kernel_spmd(
        nc, in_maps, list(range(NCORES)), trace=trace
    )

    N, O = cfg["N"], cfg["O"]
    out = np.empty((N, O), np.float32)
    local = plan["node_slot"] * P + plan["node_part"]
    for d in range(NCORES):
        sel = plan["node_dev"] == d
        out[sel] = res.results[d]["out"][local[sel]]
    return out, res


def kernel(**inputs) -> np.ndarray:
    out, _ = _run(inputs)
    return out
